# revision 4
# baseline (speedup 1.0000x reference)
"""Trainium2 Bass kernel for nn_GroupedConvFuseSide4.

out[b,k] = w[k,0]*side5[b,k] + w[k,1]*side4[b,k]
         + w[k,2]*side1[b,0] + w[k,3]*side2[b,0] + w[k,4]*side3[b,0] + bias[k]

Sharding: pure data parallel over batch (B=8) across 8 NeuronCores.
Weights/bias are baked into the program as immediates (compiled per call).
"""

import numpy as np

B, K, H, W = 8, 19, 512, 512
P = 128                    # SBUF partitions
FD = (H * W) // P          # 2048 free elems per partition for one plane
N_CORES = 8

_cache = {}


def _build_program(w, b):
    import concourse.bass as bass
    import concourse.bacc as bacc
    import concourse.tile as tile
    import concourse.mybir as mybir
    from contextlib import ExitStack

    f32 = mybir.dt.float32
    mult = mybir.AluOpType.mult
    add = mybir.AluOpType.add
    Copy = mybir.ActivationFunctionType.Copy

    nc = bacc.Bacc(
        "TRN2", target_bir_lowering=False, debug=False,
        enable_asserts=False, num_devices=N_CORES,
    )

    s1 = nc.dram_tensor("side1", [P, FD], f32, kind="ExternalInput").ap()
    s2 = nc.dram_tensor("side2", [P, FD], f32, kind="ExternalInput").ap()
    s3 = nc.dram_tensor("side3", [P, FD], f32, kind="ExternalInput").ap()
    s4 = nc.dram_tensor("side4", [K, P, FD], f32, kind="ExternalInput").ap()
    s5 = nc.dram_tensor("side5", [K, P, FD], f32, kind="ExternalInput").ap()
    out = nc.dram_tensor("out", [K, P, FD], f32, kind="ExternalOutput").ap()

    with tile.TileContext(nc) as tc, ExitStack() as ctx:
        singles = ctx.enter_context(tc.tile_pool(name="singles", bufs=1))
        in_pool = ctx.enter_context(tc.tile_pool(name="inp", bufs=3))
        tmp_pool = ctx.enter_context(tc.tile_pool(name="tmp", bufs=6))
        out_pool = ctx.enter_context(tc.tile_pool(name="outp", bufs=3))

        t1 = singles.tile([P, FD], f32, tag="s1")
        t2 = singles.tile([P, FD], f32, tag="s2")
        t3 = singles.tile([P, FD], f32, tag="s3")
        nc.sync.dma_start(out=t1[:], in_=s1)
        nc.sync.dma_start(out=t2[:], in_=s2)
        nc.sync.dma_start(out=t3[:], in_=s3)

        for k in range(K):
            t5 = in_pool.tile([P, FD], f32, tag="t5")
            nc.sync.dma_start(out=t5[:], in_=s5[k])
            t4 = in_pool.tile([P, FD], f32, tag="t4")
            nc.sync.dma_start(out=t4[:], in_=s4[k])

            # a1 = w0*s5 + bias   (ScalarE, free affine)
            a1 = tmp_pool.tile([P, FD], f32, tag="tmp")
            nc.scalar.activation(a1[:], t5[:], Copy,
                                 bias=float(b[k]), scale=float(w[k, 0]))
            # chain of scalar_tensor_tensor on DVE:
            d1 = tmp_pool.tile([P, FD], f32, tag="tmp")
            nc.vector.scalar_tensor_tensor(d1[:], t4[:], float(w[k, 1]), a1[:], mult, add)
            d2 = tmp_pool.tile([P, FD], f32, tag="tmp")
            nc.vector.scalar_tensor_tensor(d2[:], t1[:], float(w[k, 2]), d1[:], mult, add)
            d3 = tmp_pool.tile([P, FD], f32, tag="tmp")
            nc.vector.scalar_tensor_tensor(d3[:], t2[:], float(w[k, 3]), d2[:], mult, add)
            o = out_pool.tile([P, FD], f32, tag="o")
            nc.vector.scalar_tensor_tensor(o[:], t3[:], float(w[k, 4]), d3[:], mult, add)

            nc.sync.dma_start(out=out[k], in_=o[:])

    nc.compile()
    return nc


def _get_program(w, b):
    key = (w.tobytes(), b.tobytes())
    if key not in _cache:
        _cache[key] = _build_program(w, b)
    return _cache[key]


def run(inputs, trace=False, tmpdir=None):
    from concourse.bass_utils import run_bass_kernel_spmd

    w = np.asarray(inputs["weight"], dtype=np.float32)
    b = np.asarray(inputs["bias"], dtype=np.float32)
    nc = _get_program(w, b)

    in_maps = []
    for core in range(N_CORES):
        in_maps.append({
            "side1": np.ascontiguousarray(np.asarray(inputs["side1"])[core].reshape(P, FD)),
            "side2": np.ascontiguousarray(np.asarray(inputs["side2"])[core].reshape(P, FD)),
            "side3": np.ascontiguousarray(np.asarray(inputs["side3"])[core].reshape(P, FD)),
            "side4": np.ascontiguousarray(np.asarray(inputs["side4"])[core].reshape(K, P, FD)),
            "side5": np.ascontiguousarray(np.asarray(inputs["side5"])[core].reshape(K, P, FD)),
        })

    res = run_bass_kernel_spmd(nc, in_maps, list(range(N_CORES)),
                               trace=trace, tmpdir=tmpdir)
    outs = [res.results[i]["out"].reshape(1, K, H, W) for i in range(N_CORES)]
    return np.concatenate(outs, axis=0), res


def kernel(**inputs):
    out, _ = run(inputs, trace=False)
    return out
